# revision 23
# baseline (speedup 1.0000x reference)
import sys
sys.path.insert(0, '/opt/trn_rl_repo')
import numpy as np
import ml_dtypes

import concourse.bass as bass
from concourse import bacc
import concourse.mybir as mybir
import concourse.tile_utils as tile_utils
tile_utils.max_sbuf_usage = 234 * 1024
from concourse.tile import TileContext
from concourse.bass_utils import run_bass_kernel_spmd

F32 = mybir.dt.float32
BF16 = mybir.dt.bfloat16
AF = mybir.ActivationFunctionType
ALU = mybir.AluOpType
BF = ml_dtypes.bfloat16

B, S, D, H, DFF = 16, 512, 512, 8, 2048
L = 6
NCORES = 8
BL = B // NCORES
NEG = -1e30
NT = 4
NVS = [512 - 128 * j for j in range(NT)]
LCFG = [('y', 'y', 1, True), ('y', 'y', 1, True),
        ('x', 'x', 1, False), ('x', 'y', 0, True),
        ('x', 'x', 1, False), ('x', 'y', 0, True)]
_CACHE = {}
NLAYERS = L
DBG = False


def _build():
    nc = bacc.Bacc()
    dp = nc.declare_dram_parameter
    x0_e = dp("x0", [BL, 128, NT, 512], F32, isOutput=False)
    y0_e = dp("y0", [BL, 128, NT, 512], F32, isOutput=False)
    wk_e = dp("wk", [L, 128, NT, 512], BF16, isOutput=False)
    wv_e = dp("wv", [L, 128, NT, 512], BF16, isOutput=False)
    wo_e = dp("wo", [L, 128, NT, 512], BF16, isOutput=False)
    w1_e = dp("w1", [L, 128, NT, 2048], BF16, isOutput=False)
    w2_e = dp("w2", [L, 128, 16, 512], BF16, isOutput=False)
    bkc_e = dp("bkc", [L, 128, NT], F32, isOutput=False)
    b1c_e = dp("b1c", [L, 128, 16], F32, isOutput=False)
    l1g_e = dp("l1g", [L, 128, NT], F32, isOutput=False)
    l1b_e = dp("l1b", [L, 128, NT], F32, isOutput=False)
    l2g_e = dp("l2g", [L, 128, NT], F32, isOutput=False)
    l2b_e = dp("l2b", [L, 128, NT], F32, isOutput=False)
    bvr_e = dp("bvr", [L, 1, 512], BF16, isOutput=False)
    bor_e = dp("bor", [L, 1, 512], BF16, isOutput=False)
    b2r_e = dp("b2r", [L, 1, 512], BF16, isOutput=False)
    gam_e = dp("gam", [1, L * H], F32, isOutput=False)
    nm1_e = dp("nm1", [NT, 128, 512], BF16, isOutput=False)
    nm0_e = dp("nm0", [NT, 128, 512], BF16, isOutput=False)
    pos_e = dp("posm", [NT, 128, 512], BF16, isOutput=False)
    tri_e = dp("tril", [128, 128], BF16, isOutput=False)
    onm_e = dp("onesm", [128, 128], BF16, isOutput=False)
    idn_e = dp("idn", [128, 128], F32, isOutput=False)
    out_e = dp("out", [BL, 2, NT, 128, 512], F32, isOutput=True)
    if DBG:
        dbg_an = dp("dbg_an", [128, NT, 512], F32, isOutput=True)
        dbg_x = dp("dbg_x", [128, NT, 512], F32, isOutput=True)
        dbg_e1 = dp("dbg_e1", [128, 512], F32, isOutput=True)
        dbg_e2 = dp("dbg_e2", [128, 512], F32, isOutput=True)
        dbg_v = dp("dbg_v", [128, 512], F32, isOutput=True)

    from contextlib import ExitStack
    with TileContext(nc) as tc, ExitStack() as _st:
        cst = _st.enter_context(tc.tile_pool(name="cst", bufs=1))
        wp = _st.enter_context(tc.tile_pool(name="wp", bufs=1))
        xp = _st.enter_context(tc.tile_pool(name="xp", bufs=1))
        apl = _st.enter_context(tc.tile_pool(name="apl", bufs=2))
        held = _st.enter_context(tc.tile_pool(name="held", bufs=8))
        eph = _st.enter_context(tc.tile_pool(name="eph", bufs=2))
        hp = _st.enter_context(tc.tile_pool(name="hp", bufs=2))
        rp = _st.enter_context(tc.tile_pool(name="rp", bufs=2))
        rs = _st.enter_context(tc.tile_pool(name="rs", bufs=1))
        lr = _st.enter_context(tc.tile_pool(name="lr", bufs=1))
        wq = _st.enter_context(tc.tile_pool(name="wq", bufs=2))
        ps = _st.enter_context(tc.tile_pool(name="ps", bufs=5, space="PSUM"))
        psb = _st.enter_context(tc.tile_pool(name="psb", bufs=2, space="PSUM"))
        psr = _st.enter_context(tc.tile_pool(name="psr", bufs=1, space="PSUM"))
        dma = nc.default_dma_engine

        nmt = {}
        posm = []
        for j in range(NT):
            nv, ic = NVS[j], 128 * j
            t1 = cst.tile([128, nv], BF16, tag=f"nm1_{j}")
            t0 = cst.tile([128, nv], BF16, tag=f"nm0_{j}")
            pm = cst.tile([128, nv], BF16, tag=f"pos_{j}")
            dma.dma_start(out=t1, in_=nm1_e[j, :, ic:512])
            dma.dma_start(out=t0, in_=nm0_e[j, :, ic:512])
            dma.dma_start(out=pm, in_=pos_e[j, :, ic:512])
            nmt[(1, j)] = t1
            nmt[(0, j)] = t0
            posm.append(pm)
        tril = cst.tile([128, 128], BF16, tag="tril")
        onesm = cst.tile([128, 128], BF16, tag="onesm")
        idn = cst.tile([128, 128], F32, tag="idn")
        dma.dma_start(out=tril, in_=tri_e[:])
        dma.dma_start(out=onesm, in_=onm_e[:])
        dma.dma_start(out=idn, in_=idn_e[:])
        onescol = cst.tile([128, 1], BF16, tag="onescol")
        ones1x = cst.tile([1, 128], BF16, tag="ones1x")
        onesrow = cst.tile([1, 512], BF16, tag="onesrow")
        ninv512 = cst.tile([1, 512], F32, tag="ninv512")
        nc.vector.memset(onescol, 1.0)
        nc.vector.memset(ones1x, 1.0)
        nc.vector.memset(onesrow, 1.0)
        nc.vector.memset(ninv512, -1.0 / 512.0)
        one11 = cst.tile([1, 1], F32, tag="one11")
        eps11 = cst.tile([1, 1], F32, tag="eps11")
        nc.vector.memset(one11, 1.0)
        nc.vector.memset(eps11, 1e-5)

        gam = cst.tile([1, L * H], F32, tag="gam")
        dma.dma_start(out=gam, in_=gam_e[:])
        ge = cst.tile([1, L * H], F32, tag="ge")
        nc.scalar.activation(out=ge, in_=gam, func=AF.Exp)
        gl = cst.tile([1, L * H], F32, tag="gl")
        nc.scalar.activation(out=gl, in_=ge, func=AF.Ln, bias=one11[:])
        gn = cst.tile([1, L * H], F32, tag="gn")
        nc.vector.tensor_scalar_mul(gn, gl, -1.0)
        mspg = cst.tile([128, L * H], F32, tag="mspg")
        nc.gpsimd.partition_broadcast(mspg[:], gn[:])

        xT = {'x': [], 'y': []}
        x16 = {'x': [], 'y': []}
        for b in range(BL):
            for st, src in (('x', x0_e), ('y', y0_e)):
                t = xp.tile([128, NT, 512], F32, tag=f"xT_{st}_{b}")
                dma.dma_start(out=t, in_=src[b])
                c = xp.tile([128, NT, 512], BF16, tag=f"x16_{st}_{b}")
                for tt in range(NT):
                    nc.vector.tensor_copy(out=c[:, tt, :], in_=t[:, tt, :])
                xT[st].append(t)
                x16[st].append(c)

        def layer_norm(b, pss, gcol, bcol, mu_bank, xtag, x16tag):
            # pss: list of 4 PSUM tiles holding centered x (mu already subtracted)
            # var = colsum(square)/512 ; returns (xnew f32, xnew16 bf16)
            sqs = []
            for et in range(NT):
                sq = hp.tile([128, 512], BF16, tag="sq")
                nc.scalar.activation(out=sq, in_=pss[et][:], func=AF.Square)
                sqs.append(sq)
            for et in range(NT):
                nc.tensor.matmul(out=mu_bank[32:33, :], lhsT=onescol[:], rhs=sqs[et][:],
                                 start=(et == 0), stop=(et == NT - 1))
            xc = []
            for et in range(NT):
                t = hp.tile([128, 512], F32, tag="xc")
                if et % 2 == 0:
                    nc.scalar.activation(out=t, in_=pss[et][:], func=AF.Copy)
                else:
                    nc.vector.tensor_copy(out=t, in_=pss[et][:])
                xc.append(t)
            r1 = rs.tile([1, 512], F32, tag="r1")
            nc.scalar.activation(out=r1, in_=mu_bank[32:33, :], func=AF.Ln,
                                 scale=1.0 / 512.0, bias=eps11[:])
            nc.scalar.activation(out=r1[:], in_=r1[:], func=AF.Exp, scale=-0.5)
            rstd16 = rs.tile([1, 512], BF16, tag="rstd16")
            nc.vector.tensor_copy(out=rstd16, in_=r1[:])
            rbl = hp.tile([128, 512], BF16, tag="rb")
            nc.gpsimd.partition_broadcast(rbl[:], rstd16[:])
            xo = xp.tile([128, NT, 512], F32, tag=xtag)
            xo16 = xp.tile([128, NT, 512], BF16, tag=x16tag)
            for et in range(NT):
                nc.vector.scalar_tensor_tensor(out=xo[:, et, :], in0=xc[et][:], scalar=1.0,
                                               op0=ALU.mult, in1=rbl[:], op1=ALU.mult)
                nc.scalar.activation(out=xo[:, et, :], in_=xo[:, et, :], func=AF.Identity,
                                     bias=bcol[:, et:et + 1], scale=gcol[:, et:et + 1])
                nc.vector.tensor_copy(out=xo16[:, et, :], in_=xo[:, et, :])
            return xo, xo16

        for l in range(NLAYERS):
            qs, vs, mf, ffn = LCFG[l]
            wk = wp.tile([128, NT, 512], BF16, tag="wk")
            wv = wp.tile([128, NT, 512], BF16, tag="wv")
            wo = wp.tile([128, NT, 512], BF16, tag="wo")
            dma.dma_start(out=wk, in_=wk_e[l])
            dma.dma_start(out=wv, in_=wv_e[l])
            dma.dma_start(out=wo, in_=wo_e[l])
            bkc = lr.tile([128, NT], F32, tag="bkc")
            dma.dma_start(out=bkc, in_=bkc_e[l])
            bvr = lr.tile([1, 512], BF16, tag="bvr")
            bor = lr.tile([1, 512], BF16, tag="bor")
            dma.dma_start(out=bvr, in_=bvr_e[l])
            dma.dma_start(out=bor, in_=bor_e[l])
            l1g = lr.tile([128, NT], F32, tag="l1g")
            l1b = lr.tile([128, NT], F32, tag="l1b")
            dma.dma_start(out=l1g, in_=l1g_e[l])
            dma.dma_start(out=l1b, in_=l1b_e[l])
            if ffn:
                b1c = lr.tile([128, 16], F32, tag="b1c")
                dma.dma_start(out=b1c, in_=b1c_e[l])
                b2r = lr.tile([1, 512], BF16, tag="b2r")
                dma.dma_start(out=b2r, in_=b2r_e[l])
                l2g = lr.tile([128, NT], F32, tag="l2g")
                l2b = lr.tile([128, NT], F32, tag="l2b")
                dma.dma_start(out=l2g, in_=l2g_e[l])
                dma.dma_start(out=l2b, in_=l2b_e[l])
                sb2 = lr.tile([1, 1], F32, tag="sb2")
                nc.vector.tensor_reduce(out=sb2, in_=b2r[:], axis=mybir.AxisListType.X,
                                        op=ALU.add)

            wobf = lr.tile([128, NT], F32, tag="wobf")
            for ct in range(NT):
                nc.vector.tensor_reduce(out=wobf[:, ct:ct + 1], in_=wo[:, ct, :],
                                        axis=mybir.AxisListType.X, op=ALU.add)
            wob16 = rs.tile([128, NT], BF16, tag="wob16")
            nc.vector.tensor_copy(out=wob16, in_=wobf)
            sbo = lr.tile([1, 1], F32, tag="sbo")
            nc.vector.tensor_reduce(out=sbo, in_=bor[:], axis=mybir.AxisListType.X,
                                    op=ALU.add)

            for b in range(BL):
                qin16 = x16[qs][b]
                vin16 = x16[vs][b]
                qk16 = apl.tile([128, NT, 512], BF16, tag="qk16")
                for et in range(NT):
                    pq = psb.tile([128, 512], F32, tag="pp")
                    for ct in range(NT):
                        nc.tensor.matmul(out=pq[:], lhsT=wk[:, ct, 128 * et:128 * et + 128],
                                         rhs=qin16[:, ct, :], start=(ct == 0), stop=(ct == NT - 1))
                    nc.scalar.activation(out=qk16[:, et, :], in_=pq[:], func=AF.Identity,
                                         bias=bkc[:, et:et + 1], scale=1.0)
                vh16 = apl.tile([128, NT, 512], BF16, tag="vh16")
                for jt in range(NT):
                    pv = psb.tile([128, 512], F32, tag="pp")
                    for ct in range(NT):
                        nc.tensor.matmul(out=pv[:], lhsT=vin16[:, ct, 128 * jt:128 * jt + 128],
                                         rhs=wv[:, ct, :], start=(ct == 0), stop=False)
                    nc.tensor.matmul(out=pv[:], lhsT=ones1x[:], rhs=bvr[:], start=False, stop=True)
                    nc.scalar.activation(out=vh16[:, jt, :], in_=pv[:], func=AF.Copy)

                # phase A
                sms, v16s = [], []
                TallA = rp.tile([128, 512], F32, tag="TallA")
                TallB = rp.tile([128, 512], F32, tag="TallB")
                Talls = (TallA, TallB)
                for h in range(H):
                    et, ro = h // 2, 64 * (h % 2)
                    e1h, smh, v16h = [], [], []
                    for jt in range(NT):
                        nv, ic = NVS[jt], 128 * jt
                        sps = ps.tile([128, 512], F32, tag="big")
                        nc.tensor.matmul(out=sps[:, 0:nv],
                                         lhsT=qk16[ro:ro + 64, et, 128 * jt:128 * jt + 128],
                                         rhs=qk16[ro:ro + 64, et, ic:512],
                                         start=True, stop=True)
                        sm = held.tile([128, nv], BF16, tag=f"sm{jt}")
                        nc.vector.scalar_tensor_tensor(out=sm[:], in0=sps[:, 0:nv], scalar=0.125,
                                                       in1=nmt[(mf, jt)][:],
                                                       op0=ALU.mult, op1=ALU.add)
                        e1 = eph.tile([128, nv], BF16, tag=f"e1{jt}")
                        nc.scalar.activation(out=e1, in_=sm, func=AF.Exp)
                        e1h.append(e1)
                        smh.append(sm)
                    v16h = []
                    for jt in range(NT):
                        nv, ic = NVS[jt], 128 * jt
                        Ep = ps.tile([128, 512], F32, tag="big")
                        for a in range(jt, NT):
                            nc.tensor.matmul(out=Ep[:, 128 * a:512],
                                             lhsT=(tril if a == jt else onesm)[:],
                                             rhs=e1h[a][:], start=(a == jt), stop=(a == NT - 1))
                        if jt == 0:
                            _tt = Talls[h // 4]
                            _pp = 32 * (h % 4)
                            nc.vector.tensor_tensor(out=_tt[_pp:_pp + 1, :], in0=Ep[0:1, 0:512],
                                                    in1=e1h[0][0:1, :], op=ALU.add)
                        v16 = held.tile([128, nv], BF16, tag=f"v16{jt}")
                        nc.vector.tensor_tensor(out=v16[:], in0=Ep[:, ic:512],
                                                in1=posm[jt][:], op=ALU.mult)
                        v16h.append(v16)
                    if DBG and l == 0 and b == 0 and h == 0:
                        _d = hp.tile([128, 512], F32, tag="xc")
                        nc.vector.tensor_copy(out=_d[:], in_=e1h[0][:])
                        dma.dma_start(out=dbg_e1[:], in_=_d[:])
                        _d2 = hp.tile([128, 512], F32, tag="xc")
                        nc.vector.tensor_copy(out=_d2[:], in_=v16h[0][:])
                        dma.dma_start(out=dbg_v[:], in_=_d2[:])
                    sms.append(smh)
                    v16s.append(v16h)
                # phase B (sqrt table) -- in-place sqrt
                for h in range(H):
                    for jt in range(NT):
                        nc.scalar.activation(out=v16s[h][jt][:], in_=v16s[h][jt][:], func=AF.Sqrt)
                nc.scalar.activation(out=TallA[:], in_=TallA[:], func=AF.Sqrt)
                nc.scalar.activation(out=TallB[:], in_=TallB[:], func=AF.Sqrt)
                # phase C (exp table)
                rT16s = []
                for _tt in Talls:
                    nc.vector.tensor_scalar_add(_tt[:], _tt[:], 1e-20)
                    _rT = rp.tile([128, 512], F32, tag="rT", name=f"rT_{id(_tt) % 97}")
                    nc.vector.reciprocal_approx_fast(out=_rT, in_=_tt[:])
                    rT16s.append(_rT)
                an16 = apl.tile([128, NT, 512], BF16, tag="an16")
                Zpair = None
                for h in range(H):
                    et = h // 2
                    rb = hp.tile([128, 512], BF16, tag="rb")
                    _r16 = rT16s[h // 4]
                    _pp = 32 * (h % 4)
                    _rrow = rs.tile([1, 512], BF16, tag="rTrow")
                    nc.vector.tensor_copy(out=_rrow[:], in_=_r16[_pp:_pp + 1, :])
                    nc.gpsimd.partition_broadcast(rb[:], _rrow[0:1, :])
                    e2h = []
                    for jt in range(NT):
                        nv, ic = NVS[jt], 128 * jt
                        u2 = v16s[h][jt]
                        nc.vector.tensor_tensor(out=u2[:], in0=u2[:], in1=rb[:, ic:512],
                                                op=ALU.mult)
                        nc.scalar.activation(out=u2[:], in_=u2[:], func=AF.Exp,
                                             scale=mspg[:, l * H + h:l * H + h + 1])
                        m2 = hp.tile([128, 512], BF16, tag="sq")
                        nc.vector.tensor_tensor(out=m2[:, 0:nv], in0=sms[h][jt], in1=u2[:],
                                                op=ALU.mult)
                        e2 = hp.tile([128, nv], BF16, tag=f"e2{jt}")
                        nc.scalar.activation(out=e2, in_=m2[:, 0:nv], func=AF.Exp)
                        e2h.append(e2)
                    Zp = psr.tile([64, 512], F32, tag="mu")
                    for a in range(NT):
                        nc.tensor.matmul(out=Zp[0:1, 128 * a:512], lhsT=onescol[:],
                                         rhs=e2h[a][:], start=(a == 0), stop=(a == NT - 1))
                    if h % 2 == 0:
                        Zpair = rs.tile([64, 512], F32, tag="Zpair")
                        atp = ps.tile([128, 512], F32, tag="big")
                    nc.vector.tensor_copy(out=Zpair[32 * (h % 2):32 * (h % 2) + 1, :], in_=Zp[0:1, :])
                    ro = 64 * (h % 2)
                    for a in range(NT):
                        nc.tensor.matmul(out=atp[ro:ro + 64, 128 * a:512],
                                         lhsT=vh16[:, a, 64 * h:64 * h + 64],
                                         rhs=e2h[a][:],
                                         start=(a == 0), stop=(a == NT - 1))
                    if DBG and l == 0 and b == 0 and h == 0:
                        _d3 = hp.tile([128, 512], F32, tag="xc")
                        nc.vector.tensor_copy(out=_d3[:], in_=e2h[0][:])
                        dma.dma_start(out=dbg_e2[:], in_=_d3[:])
                    if h % 2 == 1:
                        nc.vector.tensor_scalar_add(Zpair[:], Zpair[:], 1e-30)
                        rZ = rs.tile([64, 512], F32, tag="rZ")
                        nc.vector.reciprocal_approx_fast(out=rZ, in_=Zpair[:])
                        for par in range(2):
                            zr16 = rs.tile([1, 512], BF16, tag="zr")
                            nc.vector.tensor_copy(out=zr16[:], in_=rZ[32 * par:32 * par + 1, :])
                            rzb = hp.tile([128, 512], BF16, tag="rb")
                            nc.gpsimd.partition_broadcast(rzb[:], zr16[0:1, :])
                            rr = 64 * par
                            nc.vector.tensor_tensor(out=an16[rr:rr + 64, et, :],
                                                    in0=atp[rr:rr + 64, :],
                                                    in1=rzb[rr:rr + 64, :], op=ALU.mult)

                # Wo + bo + residual + (-mu) -> centered x in PSUM
                mup = psr.tile([64, 512], F32, tag="mu")
                for ct in range(NT):
                    nc.tensor.matmul(out=mup[0:1, :], lhsT=wob16[:, ct:ct + 1],
                                     rhs=an16[:, ct, :], start=(ct == 0), stop=False)
                for ct in range(NT):
                    nc.tensor.matmul(out=mup[0:1, :], lhsT=onescol[:], rhs=qin16[:, ct, :],
                                     start=False, stop=(ct == NT - 1))
                nmu = rs.tile([1, 512], BF16, tag="nmu")
                nc.vector.scalar_tensor_tensor(out=nmu, in0=mup[0:1, :], scalar=sbo[:],
                                               op0=ALU.add, in1=ninv512[:], op1=ALU.mult)
                o3p = []
                for et in range(NT):
                    op_ = ps.tile([128, 512], F32, tag="big")
                    for ct in range(NT):
                        nc.tensor.matmul(out=op_[:], lhsT=wo[:, ct, 128 * et:128 * et + 128],
                                         rhs=an16[:, ct, :], start=(ct == 0), stop=False)
                    nc.tensor.matmul(out=op_[:], lhsT=bor[0:1, 128 * et:128 * et + 128],
                                     rhs=onesrow[:], start=False, stop=False)
                    nc.tensor.matmul(out=op_[:], lhsT=idn[:], rhs=xT[qs][b][:, et, :],
                                     start=False, stop=False)
                    nc.tensor.matmul(out=op_[:], lhsT=ones1x[:], rhs=nmu[:], start=False, stop=True)
                    o3p.append(op_)
                if not ffn:
                    xn, xn16 = layer_norm(b, o3p, l1g, l1b, mup, f"xT_{qs}_{b}", f"x16_{qs}_{b}")
                    xT[qs][b] = xn
                    x16[qs][b] = xn16
                    continue
                xn, xn16 = layer_norm(b, o3p, l1g, l1b, mup, "ln1", "ln1b")
                if DBG and l == 0 and b == 0:
                    for _t in range(NT):
                        _d4 = hp.tile([128, 512], F32, tag="xc")
                        nc.vector.tensor_copy(out=_d4[:], in_=an16[:, _t, :])
                        dma.dma_start(out=dbg_an[:, _t, :].opt() if False else dbg_an[0:128, _t, :], in_=_d4[:])
                        nc.vector.tensor_copy(out=_d4[:], in_=xn[:, _t, :])
                        dma.dma_start(out=dbg_x[0:128, _t, :], in_=_d4[:])

                # FFN (hidden + weights streamed per quarter)
                mu2 = psr.tile([64, 512], F32, tag="mu")
                o4p = [ps.tile([128, 512], F32, tag="big", name=f"o4p{_i}") for _i in range(NT)]
                for fq in range(8):
                    w1q = wq.tile([128, NT, 256], BF16, tag="w1q")
                    dma.dma_start(out=w1q, in_=w1_e[l, :, :, 256 * fq:256 * fq + 256])
                    w2q = wq.tile([128, 2, 512], BF16, tag="w2q")
                    dma.dma_start(out=w2q, in_=w2_e[l, :, 2 * fq:2 * fq + 2, :])
                    w2bq = rs.tile([128, 2], F32, tag="w2bq")
                    for fi in range(2):
                        nc.vector.tensor_reduce(out=w2bq[:, fi:fi + 1], in_=w2q[:, fi, :],
                                                axis=mybir.AxisListType.X, op=ALU.add)
                    w2bq16 = rs.tile([128, 2], BF16, tag="w2bq16")
                    nc.vector.tensor_copy(out=w2bq16, in_=w2bq)
                    for fi in range(2):
                        ft = 2 * fq + fi
                        php = psb.tile([128, 512], F32, tag="pp")
                        for ct in range(NT):
                            nc.tensor.matmul(out=php[:], lhsT=w1q[:, ct, 128 * fi:128 * fi + 128],
                                             rhs=xn16[:, ct, :], start=(ct == 0), stop=(ct == NT - 1))
                        h16f = hp.tile([128, 512], BF16, tag="h16f")
                        nc.scalar.activation(out=h16f, in_=php[:], func=AF.Relu,
                                             bias=b1c[:, ft:ft + 1], scale=1.0)
                        nc.tensor.matmul(out=mu2[0:1, :], lhsT=w2bq16[:, fi:fi + 1],
                                         rhs=h16f[:], start=(ft == 0), stop=False)
                        for et in range(NT):
                            nc.tensor.matmul(out=o4p[et][:], lhsT=w2q[:, fi, 128 * et:128 * et + 128],
                                             rhs=h16f[:], start=(ft == 0), stop=False)
                for ct in range(NT):
                    nc.tensor.matmul(out=mu2[0:1, :], lhsT=onescol[:], rhs=xn16[:, ct, :],
                                     start=False, stop=(ct == NT - 1))
                nmu2 = rs.tile([1, 512], BF16, tag="nmu")
                nc.vector.scalar_tensor_tensor(out=nmu2, in0=mu2[0:1, :], scalar=sb2[:],
                                               op0=ALU.add, in1=ninv512[:], op1=ALU.mult)
                for et in range(NT):
                    nc.tensor.matmul(out=o4p[et][:], lhsT=b2r[0:1, 128 * et:128 * et + 128],
                                     rhs=onesrow[:], start=False, stop=False)
                    nc.tensor.matmul(out=o4p[et][:], lhsT=idn[:], rhs=xn[:, et, :],
                                     start=False, stop=False)
                    nc.tensor.matmul(out=o4p[et][:], lhsT=ones1x[:], rhs=nmu2[:], start=False, stop=True)
                xo, xo16 = layer_norm(b, o4p, l2g, l2b, mu2, f"xT_{qs}_{b}", f"x16_{qs}_{b}")
                xT[qs][b] = xo
                x16[qs][b] = xo16

        for b in range(BL):
            for oi, st in enumerate(['x', 'y']):
                src = xT[st][b]
                for it in range(NT):
                    tok = hp.tile([128, 512], F32, tag="xc")
                    for et in range(NT):
                        pt = psb.tile([128, 128], F32, tag="pp")
                        nc.tensor.transpose(out=pt[:], in_=src[:, et, 128 * it:128 * it + 128],
                                            identity=idn[:])
                        nc.scalar.activation(out=tok[:, 128 * et:128 * et + 128], in_=pt[:],
                                             func=AF.Copy)
                    dma.dma_start(out=out_e[b, oi, it], in_=tok[:])

    nc.compile()
    return nc


def _prep(inputs):
    f32 = np.float32
    q = np.asarray(inputs['q_embed_data'], f32)
    qa = np.asarray(inputs['qa_embed_data'], f32)

    def fmaj(x):
        # [L, R, C] -> [L, 128, R//128, C] partition-major on R
        Lx, R, C = x.shape
        return np.ascontiguousarray(x.reshape(Lx, R // 128, 128, C).transpose(0, 2, 1, 3))

    def cols(v):
        return np.ascontiguousarray(np.asarray(v, f32).reshape(L, -1, 128).transpose(0, 2, 1))

    jj = np.arange(S)[:, None]   # j (partition/rows)
    ii = np.arange(S)[None, :]   # i (free/cols)
    nm1 = np.where(jj <= ii, 0.0, NEG).astype(BF).reshape(NT, 128, S)
    nm0 = np.where(jj < ii, 0.0, NEG).astype(BF).reshape(NT, 128, S)
    posm = np.abs(ii - jj).astype(f32).astype(BF).reshape(NT, 128, S)

    base = {
        'wk': fmaj(np.asarray(inputs['Wk'], f32)).astype(BF),
        'wv': fmaj(np.asarray(inputs['Wv'], f32)).astype(BF),
        'wo': fmaj(np.asarray(inputs['Wo'], f32)).astype(BF),
        'w1': fmaj(np.asarray(inputs['W1'], f32)).astype(BF),
        'w2': fmaj(np.asarray(inputs['W2'], f32)).astype(BF),
        'bkc': cols(inputs['bk']),
        'b1c': cols(inputs['b1']),
        'l1g': cols(inputs['ln1_g']), 'l1b': cols(inputs['ln1_b']),
        'l2g': cols(inputs['ln2_g']), 'l2b': cols(inputs['ln2_b']),
        'bvr': np.asarray(inputs['bv'], f32).astype(BF).reshape(L, 1, S),
        'bor': np.asarray(inputs['bo'], f32).astype(BF).reshape(L, 1, S),
        'b2r': np.asarray(inputs['b2'], f32).astype(BF).reshape(L, 1, S),
        'gam': np.asarray(inputs['gammas'], f32).reshape(1, L * H),
        'nm1': np.ascontiguousarray(nm1), 'nm0': np.ascontiguousarray(nm0),
        'posm': np.ascontiguousarray(posm),
        'tril': np.tril(np.ones((128, 128), f32), -1).astype(BF),
        'onesm': np.ones((128, 128), f32).astype(BF),
        'idn': np.eye(128, dtype=f32),
    }
    in_maps = []
    for c in range(NCORES):
        m = dict(base)
        for k, src in (('x0', q), ('y0', qa)):
            sh = src[c * BL:(c + 1) * BL]  # [BL, S(i), D(c)]
            m[k] = np.ascontiguousarray(
                sh.transpose(0, 2, 1).reshape(BL, NT, 128, S).transpose(0, 2, 1, 3))
        in_maps.append(m)
    return in_maps


def kernel(**inputs):
    if 'nc' not in _CACHE:
        _CACHE['nc'] = _build()
    nc = _CACHE['nc']
    in_maps = _prep(inputs)
    res = run_bass_kernel_spmd(nc, in_maps, list(range(NCORES)))
    xs, ys = [], []
    for c in range(NCORES):
        o = np.asarray(res.results[c]['out'])  # [BL, 2, NT, 128, 512]
        xs.append(o[:, 0].reshape(BL, S, D))
        ys.append(o[:, 1].reshape(BL, S, D))
    return (np.concatenate(xs, 0), np.concatenate(ys, 0))



# revision 24
# speedup vs baseline: 1.1606x; 1.1606x over previous
import sys
sys.path.insert(0, '/opt/trn_rl_repo')
import numpy as np
import ml_dtypes

import concourse.bass as bass
from concourse import bacc
import concourse.mybir as mybir
import concourse.tile_utils as tile_utils
tile_utils.max_sbuf_usage = 234 * 1024
from concourse.tile import TileContext
from concourse.bass_utils import run_bass_kernel_spmd

F32 = mybir.dt.float32
BF16 = mybir.dt.bfloat16
AF = mybir.ActivationFunctionType
ALU = mybir.AluOpType
BF = ml_dtypes.bfloat16

B, S, D, H, DFF = 16, 512, 512, 8, 2048
L = 6
NCORES = 8
BL = B // NCORES
NEG = -1e30
NT = 4
NVS = [512 - 128 * j for j in range(NT)]
LCFG = [('y', 'y', 1, True), ('y', 'y', 1, True),
        ('x', 'x', 1, False), ('x', 'y', 0, True),
        ('x', 'x', 1, False), ('x', 'y', 0, True)]
_CACHE = {}
NLAYERS = L
DBG = False


def _build():
    nc = bacc.Bacc()
    dp = nc.declare_dram_parameter
    x0_e = dp("x0", [BL, 128, NT, 512], F32, isOutput=False)
    y0_e = dp("y0", [BL, 128, NT, 512], F32, isOutput=False)
    wk_e = dp("wk", [L, 128, NT, 512], BF16, isOutput=False)
    wv_e = dp("wv", [L, 128, NT, 512], BF16, isOutput=False)
    wo_e = dp("wo", [L, 128, NT, 512], BF16, isOutput=False)
    w1_e = dp("w1", [L, 128, NT, 2048], BF16, isOutput=False)
    w2_e = dp("w2", [L, 128, 16, 512], BF16, isOutput=False)
    bkc_e = dp("bkc", [L, 128, NT], F32, isOutput=False)
    b1c_e = dp("b1c", [L, 128, 16], F32, isOutput=False)
    l1g_e = dp("l1g", [L, 128, NT], F32, isOutput=False)
    l1b_e = dp("l1b", [L, 128, NT], F32, isOutput=False)
    l2g_e = dp("l2g", [L, 128, NT], F32, isOutput=False)
    l2b_e = dp("l2b", [L, 128, NT], F32, isOutput=False)
    bvr_e = dp("bvr", [L, 1, 512], BF16, isOutput=False)
    bor_e = dp("bor", [L, 1, 512], BF16, isOutput=False)
    b2r_e = dp("b2r", [L, 1, 512], BF16, isOutput=False)
    gam_e = dp("gam", [1, L * H], F32, isOutput=False)
    nm1_e = dp("nm1", [NT, 128, 512], BF16, isOutput=False)
    nm0_e = dp("nm0", [NT, 128, 512], BF16, isOutput=False)
    pos_e = dp("posm", [NT, 128, 512], BF16, isOutput=False)
    tri_e = dp("tril", [128, 128], BF16, isOutput=False)
    onm_e = dp("onesm", [128, 128], BF16, isOutput=False)
    idn_e = dp("idn", [128, 128], F32, isOutput=False)
    out_e = dp("out", [BL, 2, NT, 128, 512], F32, isOutput=True)
    if DBG:
        dbg_an = dp("dbg_an", [128, NT, 512], F32, isOutput=True)
        dbg_x = dp("dbg_x", [128, NT, 512], F32, isOutput=True)
        dbg_e1 = dp("dbg_e1", [128, 512], F32, isOutput=True)
        dbg_e2 = dp("dbg_e2", [128, 512], F32, isOutput=True)
        dbg_v = dp("dbg_v", [128, 512], F32, isOutput=True)

    from contextlib import ExitStack
    with TileContext(nc) as tc, ExitStack() as _st:
        cst = _st.enter_context(tc.tile_pool(name="cst", bufs=1))
        wp = _st.enter_context(tc.tile_pool(name="wp", bufs=1))
        xp = _st.enter_context(tc.tile_pool(name="xp", bufs=1))
        apl = _st.enter_context(tc.tile_pool(name="apl", bufs=1))
        held = _st.enter_context(tc.tile_pool(name="held", bufs=8))
        eph = _st.enter_context(tc.tile_pool(name="eph", bufs=2))
        hp = _st.enter_context(tc.tile_pool(name="hp", bufs=2))
        rp = _st.enter_context(tc.tile_pool(name="rp", bufs=2))
        rs = _st.enter_context(tc.tile_pool(name="rs", bufs=1))
        lr = _st.enter_context(tc.tile_pool(name="lr", bufs=1))
        wq = _st.enter_context(tc.tile_pool(name="wq", bufs=2))
        ps = _st.enter_context(tc.tile_pool(name="ps", bufs=5, space="PSUM"))
        psb = _st.enter_context(tc.tile_pool(name="psb", bufs=2, space="PSUM"))
        psr = _st.enter_context(tc.tile_pool(name="psr", bufs=1, space="PSUM"))
        dma = nc.default_dma_engine

        nmt = {}
        posm = []
        for j in range(NT):
            nv, ic = NVS[j], 128 * j
            t1 = cst.tile([128, nv], BF16, tag=f"nm1_{j}")
            t0 = cst.tile([128, nv], BF16, tag=f"nm0_{j}")
            pm = cst.tile([128, nv], BF16, tag=f"pos_{j}")
            dma.dma_start(out=t1, in_=nm1_e[j, :, ic:512])
            dma.dma_start(out=t0, in_=nm0_e[j, :, ic:512])
            dma.dma_start(out=pm, in_=pos_e[j, :, ic:512])
            nmt[(1, j)] = t1
            nmt[(0, j)] = t0
            posm.append(pm)
        tril = cst.tile([128, 128], BF16, tag="tril")
        onesm = cst.tile([128, 128], BF16, tag="onesm")
        idn = cst.tile([128, 128], F32, tag="idn")
        dma.dma_start(out=tril, in_=tri_e[:])
        dma.dma_start(out=onesm, in_=onm_e[:])
        dma.dma_start(out=idn, in_=idn_e[:])
        onescol = cst.tile([128, 1], BF16, tag="onescol")
        ones1x = cst.tile([1, 128], BF16, tag="ones1x")
        onesrow = cst.tile([1, 512], BF16, tag="onesrow")
        ninv512 = cst.tile([1, 512], F32, tag="ninv512")
        nc.vector.memset(onescol, 1.0)
        nc.vector.memset(ones1x, 1.0)
        nc.vector.memset(onesrow, 1.0)
        nc.vector.memset(ninv512, -1.0 / 512.0)
        one11 = cst.tile([1, 1], F32, tag="one11")
        eps11 = cst.tile([1, 1], F32, tag="eps11")
        nc.vector.memset(one11, 1.0)
        nc.vector.memset(eps11, 1e-5)

        gam = cst.tile([1, L * H], F32, tag="gam")
        dma.dma_start(out=gam, in_=gam_e[:])
        ge = cst.tile([1, L * H], F32, tag="ge")
        nc.scalar.activation(out=ge, in_=gam, func=AF.Exp)
        gl = cst.tile([1, L * H], F32, tag="gl")
        nc.scalar.activation(out=gl, in_=ge, func=AF.Ln, bias=one11[:])
        gn = cst.tile([1, L * H], F32, tag="gn")
        nc.vector.tensor_scalar_mul(gn, gl, -1.0)
        mspg = cst.tile([128, L * H], F32, tag="mspg")
        nc.gpsimd.partition_broadcast(mspg[:], gn[:])

        xT = {'x': [], 'y': []}
        x16 = {'x': [], 'y': []}
        for b in range(BL):
            for st, src in (('x', x0_e), ('y', y0_e)):
                t = xp.tile([128, NT, 512], F32, tag=f"xT_{st}_{b}")
                dma.dma_start(out=t, in_=src[b])
                c = xp.tile([128, NT, 512], BF16, tag=f"x16_{st}_{b}")
                for tt in range(NT):
                    nc.vector.tensor_copy(out=c[:, tt, :], in_=t[:, tt, :])
                xT[st].append(t)
                x16[st].append(c)

        def layer_norm(b, pss, gcol, bcol, mu_bank, xtag, x16tag):
            # pss: list of 4 PSUM tiles holding centered x (mu already subtracted)
            # var = colsum(square)/512 ; returns (xnew f32, xnew16 bf16)
            sqs = []
            for et in range(NT):
                sq = hp.tile([128, 512], BF16, tag="sq")
                nc.scalar.activation(out=sq, in_=pss[et][:], func=AF.Square)
                sqs.append(sq)
            for et in range(NT):
                nc.tensor.matmul(out=mu_bank[32:33, :], lhsT=onescol[:], rhs=sqs[et][:],
                                 start=(et == 0), stop=(et == NT - 1))
            xc = []
            for et in range(NT):
                t = hp.tile([128, 512], F32, tag="xc")
                if et % 2 == 0:
                    nc.scalar.activation(out=t, in_=pss[et][:], func=AF.Copy)
                else:
                    nc.vector.tensor_copy(out=t, in_=pss[et][:])
                xc.append(t)
            r1 = rs.tile([1, 512], F32, tag="r1")
            nc.scalar.activation(out=r1, in_=mu_bank[32:33, :], func=AF.Ln,
                                 scale=1.0 / 512.0, bias=eps11[:])
            nc.scalar.activation(out=r1[:], in_=r1[:], func=AF.Exp, scale=-0.5)
            rstd16 = rs.tile([1, 512], BF16, tag="rstd16")
            nc.vector.tensor_copy(out=rstd16, in_=r1[:])
            rbl = hp.tile([128, 512], BF16, tag="rb")
            nc.gpsimd.partition_broadcast(rbl[:], rstd16[:])
            xo = xp.tile([128, NT, 512], F32, tag=xtag)
            xo16 = xp.tile([128, NT, 512], BF16, tag=x16tag)
            for et in range(NT):
                nc.vector.scalar_tensor_tensor(out=xo[:, et, :], in0=xc[et][:], scalar=1.0,
                                               op0=ALU.mult, in1=rbl[:], op1=ALU.mult)
                nc.scalar.activation(out=xo[:, et, :], in_=xo[:, et, :], func=AF.Identity,
                                     bias=bcol[:, et:et + 1], scale=gcol[:, et:et + 1])
                nc.vector.tensor_copy(out=xo16[:, et, :], in_=xo[:, et, :])
            return xo, xo16

        for l in range(NLAYERS):
            qs, vs, mf, ffn = LCFG[l]
            wk = wp.tile([128, NT, 512], BF16, tag="wk")
            wv = wp.tile([128, NT, 512], BF16, tag="wv")
            wo = wp.tile([128, NT, 512], BF16, tag="wo")
            dma.dma_start(out=wk, in_=wk_e[l])
            dma.dma_start(out=wv, in_=wv_e[l])
            dma.dma_start(out=wo, in_=wo_e[l])
            bkc = lr.tile([128, NT], F32, tag="bkc")
            dma.dma_start(out=bkc, in_=bkc_e[l])
            bvr = lr.tile([1, 512], BF16, tag="bvr")
            bor = lr.tile([1, 512], BF16, tag="bor")
            dma.dma_start(out=bvr, in_=bvr_e[l])
            dma.dma_start(out=bor, in_=bor_e[l])
            l1g = lr.tile([128, NT], F32, tag="l1g")
            l1b = lr.tile([128, NT], F32, tag="l1b")
            dma.dma_start(out=l1g, in_=l1g_e[l])
            dma.dma_start(out=l1b, in_=l1b_e[l])
            if ffn:
                b1c = lr.tile([128, 16], F32, tag="b1c")
                dma.dma_start(out=b1c, in_=b1c_e[l])
                b2r = lr.tile([1, 512], BF16, tag="b2r")
                dma.dma_start(out=b2r, in_=b2r_e[l])
                l2g = lr.tile([128, NT], F32, tag="l2g")
                l2b = lr.tile([128, NT], F32, tag="l2b")
                dma.dma_start(out=l2g, in_=l2g_e[l])
                dma.dma_start(out=l2b, in_=l2b_e[l])
                sb2 = lr.tile([1, 1], F32, tag="sb2")
                nc.vector.tensor_reduce(out=sb2, in_=b2r[:], axis=mybir.AxisListType.X,
                                        op=ALU.add)

            wobf = lr.tile([128, NT], F32, tag="wobf")
            for ct in range(NT):
                nc.vector.tensor_reduce(out=wobf[:, ct:ct + 1], in_=wo[:, ct, :],
                                        axis=mybir.AxisListType.X, op=ALU.add)
            wob16 = rs.tile([128, NT], BF16, tag="wob16")
            nc.vector.tensor_copy(out=wob16, in_=wobf)
            sbo = lr.tile([1, 1], F32, tag="sbo")
            nc.vector.tensor_reduce(out=sbo, in_=bor[:], axis=mybir.AxisListType.X,
                                    op=ALU.add)

            for b in range(BL):
                qin16 = x16[qs][b]
                vin16 = x16[vs][b]
                qk16 = apl.tile([128, NT, 512], BF16, tag="qk16")
                for et in range(NT):
                    pq = psb.tile([128, 512], F32, tag="pp")
                    for ct in range(NT):
                        nc.tensor.matmul(out=pq[:], lhsT=wk[:, ct, 128 * et:128 * et + 128],
                                         rhs=qin16[:, ct, :], start=(ct == 0), stop=(ct == NT - 1))
                    nc.scalar.activation(out=qk16[:, et, :], in_=pq[:], func=AF.Identity,
                                         bias=bkc[:, et:et + 1], scale=1.0)
                vh16 = apl.tile([128, NT, 512], BF16, tag="vh16")
                for jt in range(NT):
                    pv = psb.tile([128, 512], F32, tag="pp")
                    for ct in range(NT):
                        nc.tensor.matmul(out=pv[:], lhsT=vin16[:, ct, 128 * jt:128 * jt + 128],
                                         rhs=wv[:, ct, :], start=(ct == 0), stop=False)
                    nc.tensor.matmul(out=pv[:], lhsT=ones1x[:], rhs=bvr[:], start=False, stop=True)
                    nc.scalar.activation(out=vh16[:, jt, :], in_=pv[:], func=AF.Copy)

                # phase A
                sms, v16s = [], []
                TallA = rp.tile([128, 512], F32, tag="TallA")
                TallB = rp.tile([128, 512], F32, tag="TallB")
                Talls = (TallA, TallB)
                for h in range(H):
                    et, ro = h // 2, 64 * (h % 2)
                    e1h, smh, v16h = [], [], []
                    for jt in range(NT):
                        nv, ic = NVS[jt], 128 * jt
                        sps = ps.tile([128, 512], F32, tag="big")
                        nc.tensor.matmul(out=sps[:, 0:nv],
                                         lhsT=qk16[ro:ro + 64, et, 128 * jt:128 * jt + 128],
                                         rhs=qk16[ro:ro + 64, et, ic:512],
                                         start=True, stop=True)
                        sm = held.tile([128, nv], BF16, tag=f"sm{jt}")
                        nc.vector.scalar_tensor_tensor(out=sm[:], in0=sps[:, 0:nv], scalar=0.125,
                                                       in1=nmt[(mf, jt)][:],
                                                       op0=ALU.mult, op1=ALU.add)
                        e1 = eph.tile([128, nv], BF16, tag=f"e1{jt}")
                        nc.scalar.activation(out=e1, in_=sm, func=AF.Exp)
                        e1h.append(e1)
                        smh.append(sm)
                    v16h = []
                    for jt in range(NT):
                        nv, ic = NVS[jt], 128 * jt
                        Ep = ps.tile([128, 512], F32, tag="big")
                        for a in range(jt, NT):
                            nc.tensor.matmul(out=Ep[:, 128 * a:512],
                                             lhsT=(tril if a == jt else onesm)[:],
                                             rhs=e1h[a][:], start=(a == jt), stop=(a == NT - 1))
                        if jt == 0:
                            _tt = Talls[h // 4]
                            _pp = 32 * (h % 4)
                            nc.vector.tensor_tensor(out=_tt[_pp:_pp + 1, :], in0=Ep[0:1, 0:512],
                                                    in1=e1h[0][0:1, :], op=ALU.add)
                        v16 = held.tile([128, nv], BF16, tag=f"v16{jt}")
                        nc.vector.tensor_tensor(out=v16[:], in0=Ep[:, ic:512],
                                                in1=posm[jt][:], op=ALU.mult)
                        v16h.append(v16)
                    if DBG and l == 0 and b == 0 and h == 0:
                        _d = hp.tile([128, 512], F32, tag="xc")
                        nc.vector.tensor_copy(out=_d[:], in_=e1h[0][:])
                        dma.dma_start(out=dbg_e1[:], in_=_d[:])
                        _d2 = hp.tile([128, 512], F32, tag="xc")
                        nc.vector.tensor_copy(out=_d2[:], in_=v16h[0][:])
                        dma.dma_start(out=dbg_v[:], in_=_d2[:])
                    sms.append(smh)
                    v16s.append(v16h)
                # phase B (sqrt table) -- in-place sqrt
                for h in range(H):
                    for jt in range(NT):
                        nc.scalar.activation(out=v16s[h][jt][:], in_=v16s[h][jt][:], func=AF.Sqrt)
                nc.scalar.activation(out=TallA[:], in_=TallA[:], func=AF.Sqrt)
                nc.scalar.activation(out=TallB[:], in_=TallB[:], func=AF.Sqrt)
                # phase C (exp table)
                rT16s = []
                for _tt in Talls:
                    nc.vector.tensor_scalar_add(_tt[:], _tt[:], 1e-20)
                    _rT = rp.tile([128, 512], F32, tag="rT", name=f"rT_{id(_tt) % 97}")
                    nc.vector.reciprocal_approx_fast(out=_rT, in_=_tt[:])
                    rT16s.append(_rT)
                an16 = apl.tile([128, NT, 512], BF16, tag="an16")
                Zpair = None
                for h in range(H):
                    et = h // 2
                    rb = hp.tile([128, 512], BF16, tag="rb")
                    _r16 = rT16s[h // 4]
                    _pp = 32 * (h % 4)
                    _rrow = rs.tile([1, 512], BF16, tag="rTrow")
                    nc.vector.tensor_copy(out=_rrow[:], in_=_r16[_pp:_pp + 1, :])
                    nc.gpsimd.partition_broadcast(rb[:], _rrow[0:1, :])
                    e2h = []
                    for jt in range(NT):
                        nv, ic = NVS[jt], 128 * jt
                        u2 = v16s[h][jt]
                        nc.vector.tensor_tensor(out=u2[:], in0=u2[:], in1=rb[:, ic:512],
                                                op=ALU.mult)
                        nc.scalar.activation(out=u2[:], in_=u2[:], func=AF.Exp,
                                             scale=mspg[:, l * H + h:l * H + h + 1])
                        m2 = hp.tile([128, 512], BF16, tag="sq")
                        nc.vector.tensor_tensor(out=m2[:, 0:nv], in0=sms[h][jt], in1=u2[:],
                                                op=ALU.mult)
                        e2 = hp.tile([128, nv], BF16, tag=f"e2{jt}")
                        nc.scalar.activation(out=e2, in_=m2[:, 0:nv], func=AF.Exp)
                        e2h.append(e2)
                    Zp = psr.tile([64, 512], F32, tag="mu")
                    for a in range(NT):
                        nc.tensor.matmul(out=Zp[0:1, 128 * a:512], lhsT=onescol[:],
                                         rhs=e2h[a][:], start=(a == 0), stop=(a == NT - 1))
                    if h % 2 == 0:
                        Zpair = rs.tile([64, 512], F32, tag="Zpair")
                        atp = ps.tile([128, 512], F32, tag="big")
                    nc.vector.tensor_copy(out=Zpair[32 * (h % 2):32 * (h % 2) + 1, :], in_=Zp[0:1, :])
                    ro = 64 * (h % 2)
                    for a in range(NT):
                        nc.tensor.matmul(out=atp[ro:ro + 64, 128 * a:512],
                                         lhsT=vh16[:, a, 64 * h:64 * h + 64],
                                         rhs=e2h[a][:],
                                         start=(a == 0), stop=(a == NT - 1))
                    if DBG and l == 0 and b == 0 and h == 0:
                        _d3 = hp.tile([128, 512], F32, tag="xc")
                        nc.vector.tensor_copy(out=_d3[:], in_=e2h[0][:])
                        dma.dma_start(out=dbg_e2[:], in_=_d3[:])
                    if h % 2 == 1:
                        nc.vector.tensor_scalar_add(Zpair[:], Zpair[:], 1e-30)
                        rZ = rs.tile([64, 512], F32, tag="rZ")
                        nc.vector.reciprocal_approx_fast(out=rZ, in_=Zpair[:])
                        for par in range(2):
                            zr16 = rs.tile([1, 512], BF16, tag="zr")
                            nc.vector.tensor_copy(out=zr16[:], in_=rZ[32 * par:32 * par + 1, :])
                            rzb = hp.tile([128, 512], BF16, tag="rb")
                            nc.gpsimd.partition_broadcast(rzb[:], zr16[0:1, :])
                            rr = 64 * par
                            nc.vector.tensor_tensor(out=an16[rr:rr + 64, et, :],
                                                    in0=atp[rr:rr + 64, :],
                                                    in1=rzb[rr:rr + 64, :], op=ALU.mult)

                # Wo + bo + residual + (-mu) -> centered x in PSUM
                mup = psr.tile([64, 512], F32, tag="mu")
                for ct in range(NT):
                    nc.tensor.matmul(out=mup[0:1, :], lhsT=wob16[:, ct:ct + 1],
                                     rhs=an16[:, ct, :], start=(ct == 0), stop=False)
                for ct in range(NT):
                    nc.tensor.matmul(out=mup[0:1, :], lhsT=onescol[:], rhs=qin16[:, ct, :],
                                     start=False, stop=(ct == NT - 1))
                nmu = rs.tile([1, 512], BF16, tag="nmu")
                nc.vector.scalar_tensor_tensor(out=nmu, in0=mup[0:1, :], scalar=sbo[:],
                                               op0=ALU.add, in1=ninv512[:], op1=ALU.mult)
                o3p = []
                for et in range(NT):
                    op_ = ps.tile([128, 512], F32, tag="big")
                    for ct in range(NT):
                        nc.tensor.matmul(out=op_[:], lhsT=wo[:, ct, 128 * et:128 * et + 128],
                                         rhs=an16[:, ct, :], start=(ct == 0), stop=False)
                    nc.tensor.matmul(out=op_[:], lhsT=bor[0:1, 128 * et:128 * et + 128],
                                     rhs=onesrow[:], start=False, stop=False)
                    nc.tensor.matmul(out=op_[:], lhsT=idn[:], rhs=xT[qs][b][:, et, :],
                                     start=False, stop=False)
                    nc.tensor.matmul(out=op_[:], lhsT=ones1x[:], rhs=nmu[:], start=False, stop=True)
                    o3p.append(op_)
                if not ffn:
                    xn, xn16 = layer_norm(b, o3p, l1g, l1b, mup, f"xT_{qs}_{b}", f"x16_{qs}_{b}")
                    xT[qs][b] = xn
                    x16[qs][b] = xn16
                    continue
                xn, xn16 = layer_norm(b, o3p, l1g, l1b, mup, "ln1", "ln1b")
                if DBG and l == 0 and b == 0:
                    for _t in range(NT):
                        _d4 = hp.tile([128, 512], F32, tag="xc")
                        nc.vector.tensor_copy(out=_d4[:], in_=an16[:, _t, :])
                        dma.dma_start(out=dbg_an[:, _t, :].opt() if False else dbg_an[0:128, _t, :], in_=_d4[:])
                        nc.vector.tensor_copy(out=_d4[:], in_=xn[:, _t, :])
                        dma.dma_start(out=dbg_x[0:128, _t, :], in_=_d4[:])

                # FFN (hidden + weights streamed per quarter)
                mu2 = psr.tile([64, 512], F32, tag="mu")
                o4p = [ps.tile([128, 512], F32, tag="big", name=f"o4p{_i}") for _i in range(NT)]
                for fq in range(8):
                    w1q = wq.tile([128, NT, 256], BF16, tag="w1q")
                    dma.dma_start(out=w1q, in_=w1_e[l, :, :, 256 * fq:256 * fq + 256])
                    w2q = wq.tile([128, 2, 512], BF16, tag="w2q")
                    dma.dma_start(out=w2q, in_=w2_e[l, :, 2 * fq:2 * fq + 2, :])
                    w2bq = rs.tile([128, 2], F32, tag="w2bq")
                    for fi in range(2):
                        nc.vector.tensor_reduce(out=w2bq[:, fi:fi + 1], in_=w2q[:, fi, :],
                                                axis=mybir.AxisListType.X, op=ALU.add)
                    w2bq16 = rs.tile([128, 2], BF16, tag="w2bq16")
                    nc.vector.tensor_copy(out=w2bq16, in_=w2bq)
                    for fi in range(2):
                        ft = 2 * fq + fi
                        php = psb.tile([128, 512], F32, tag="pp")
                        for ct in range(NT):
                            nc.tensor.matmul(out=php[:], lhsT=w1q[:, ct, 128 * fi:128 * fi + 128],
                                             rhs=xn16[:, ct, :], start=(ct == 0), stop=(ct == NT - 1))
                        h16f = hp.tile([128, 512], BF16, tag="h16f")
                        nc.scalar.activation(out=h16f, in_=php[:], func=AF.Relu,
                                             bias=b1c[:, ft:ft + 1], scale=1.0)
                        nc.tensor.matmul(out=mu2[0:1, :], lhsT=w2bq16[:, fi:fi + 1],
                                         rhs=h16f[:], start=(ft == 0), stop=False)
                        for et in range(NT):
                            nc.tensor.matmul(out=o4p[et][:], lhsT=w2q[:, fi, 128 * et:128 * et + 128],
                                             rhs=h16f[:], start=(ft == 0), stop=False)
                for ct in range(NT):
                    nc.tensor.matmul(out=mu2[0:1, :], lhsT=onescol[:], rhs=xn16[:, ct, :],
                                     start=False, stop=(ct == NT - 1))
                nmu2 = rs.tile([1, 512], BF16, tag="nmu")
                nc.vector.scalar_tensor_tensor(out=nmu2, in0=mu2[0:1, :], scalar=sb2[:],
                                               op0=ALU.add, in1=ninv512[:], op1=ALU.mult)
                for et in range(NT):
                    nc.tensor.matmul(out=o4p[et][:], lhsT=b2r[0:1, 128 * et:128 * et + 128],
                                     rhs=onesrow[:], start=False, stop=False)
                    nc.tensor.matmul(out=o4p[et][:], lhsT=idn[:], rhs=xn[:, et, :],
                                     start=False, stop=False)
                    nc.tensor.matmul(out=o4p[et][:], lhsT=ones1x[:], rhs=nmu2[:], start=False, stop=True)
                xo, xo16 = layer_norm(b, o4p, l2g, l2b, mu2, f"xT_{qs}_{b}", f"x16_{qs}_{b}")
                xT[qs][b] = xo
                x16[qs][b] = xo16

        for b in range(BL):
            for oi, st in enumerate(['x', 'y']):
                src = xT[st][b]
                for it in range(NT):
                    tok = hp.tile([128, 512], F32, tag="xc")
                    for et in range(NT):
                        pt = psb.tile([128, 128], F32, tag="pp")
                        nc.tensor.transpose(out=pt[:], in_=src[:, et, 128 * it:128 * it + 128],
                                            identity=idn[:])
                        nc.scalar.activation(out=tok[:, 128 * et:128 * et + 128], in_=pt[:],
                                             func=AF.Copy)
                    dma.dma_start(out=out_e[b, oi, it], in_=tok[:])

    nc.compile()
    return nc


def _prep(inputs):
    f32 = np.float32
    q = np.asarray(inputs['q_embed_data'], f32)
    qa = np.asarray(inputs['qa_embed_data'], f32)

    def fmaj(x):
        # [L, R, C] -> [L, 128, R//128, C] partition-major on R
        Lx, R, C = x.shape
        return np.ascontiguousarray(x.reshape(Lx, R // 128, 128, C).transpose(0, 2, 1, 3))

    def cols(v):
        return np.ascontiguousarray(np.asarray(v, f32).reshape(L, -1, 128).transpose(0, 2, 1))

    jj = np.arange(S)[:, None]   # j (partition/rows)
    ii = np.arange(S)[None, :]   # i (free/cols)
    nm1 = np.where(jj <= ii, 0.0, NEG).astype(BF).reshape(NT, 128, S)
    nm0 = np.where(jj < ii, 0.0, NEG).astype(BF).reshape(NT, 128, S)
    posm = np.abs(ii - jj).astype(f32).astype(BF).reshape(NT, 128, S)

    base = {
        'wk': fmaj(np.asarray(inputs['Wk'], f32)).astype(BF),
        'wv': fmaj(np.asarray(inputs['Wv'], f32)).astype(BF),
        'wo': fmaj(np.asarray(inputs['Wo'], f32)).astype(BF),
        'w1': fmaj(np.asarray(inputs['W1'], f32)).astype(BF),
        'w2': fmaj(np.asarray(inputs['W2'], f32)).astype(BF),
        'bkc': cols(inputs['bk']),
        'b1c': cols(inputs['b1']),
        'l1g': cols(inputs['ln1_g']), 'l1b': cols(inputs['ln1_b']),
        'l2g': cols(inputs['ln2_g']), 'l2b': cols(inputs['ln2_b']),
        'bvr': np.asarray(inputs['bv'], f32).astype(BF).reshape(L, 1, S),
        'bor': np.asarray(inputs['bo'], f32).astype(BF).reshape(L, 1, S),
        'b2r': np.asarray(inputs['b2'], f32).astype(BF).reshape(L, 1, S),
        'gam': np.asarray(inputs['gammas'], f32).reshape(1, L * H),
        'nm1': np.ascontiguousarray(nm1), 'nm0': np.ascontiguousarray(nm0),
        'posm': np.ascontiguousarray(posm),
        'tril': np.tril(np.ones((128, 128), f32), -1).astype(BF),
        'onesm': np.ones((128, 128), f32).astype(BF),
        'idn': np.eye(128, dtype=f32),
    }
    in_maps = []
    for c in range(NCORES):
        m = dict(base)
        for k, src in (('x0', q), ('y0', qa)):
            sh = src[c * BL:(c + 1) * BL]  # [BL, S(i), D(c)]
            m[k] = np.ascontiguousarray(
                sh.transpose(0, 2, 1).reshape(BL, NT, 128, S).transpose(0, 2, 1, 3))
        in_maps.append(m)
    return in_maps


def kernel(**inputs):
    if 'nc' not in _CACHE:
        _CACHE['nc'] = _build()
    nc = _CACHE['nc']
    in_maps = _prep(inputs)
    res = run_bass_kernel_spmd(nc, in_maps, list(range(NCORES)))
    xs, ys = [], []
    for c in range(NCORES):
        o = np.asarray(res.results[c]['out'])  # [BL, 2, NT, 128, 512]
        xs.append(o[:, 0].reshape(BL, S, D))
        ys.append(o[:, 1].reshape(BL, S, D))
    return (np.concatenate(xs, 0), np.concatenate(ys, 0))



# revision 25
# speedup vs baseline: 2.7412x; 2.3619x over previous
import sys
sys.path.insert(0, '/opt/trn_rl_repo')
import numpy as np
import ml_dtypes

import concourse.bass as bass
from concourse import bacc
import concourse.mybir as mybir
import concourse.tile_utils as tile_utils
tile_utils.max_sbuf_usage = 234 * 1024
from concourse.tile import TileContext
from concourse.bass_utils import run_bass_kernel_spmd

F32 = mybir.dt.float32
BF16 = mybir.dt.bfloat16
AF = mybir.ActivationFunctionType
ALU = mybir.AluOpType
BF = ml_dtypes.bfloat16

B, S, D, H, DFF = 16, 512, 512, 8, 2048
L = 6
NCORES = 8
BL = B // NCORES
NEG = -1e30
NT = 4
NVS = [512 - 128 * j for j in range(NT)]
LCFG = [('y', 'y', 1, True), ('y', 'y', 1, True),
        ('x', 'x', 1, False), ('x', 'y', 0, True),
        ('x', 'x', 1, False), ('x', 'y', 0, True)]
_CACHE = {}
NLAYERS = L
DBG = False


def _build():
    nc = bacc.Bacc()
    dp = nc.declare_dram_parameter
    x0_e = dp("x0", [BL, 128, NT, 512], F32, isOutput=False)
    y0_e = dp("y0", [BL, 128, NT, 512], F32, isOutput=False)
    wk_e = dp("wk", [L, 128, NT, 512], BF16, isOutput=False)
    wv_e = dp("wv", [L, 128, NT, 512], BF16, isOutput=False)
    wo_e = dp("wo", [L, 128, NT, 512], BF16, isOutput=False)
    w1_e = dp("w1", [L, 128, NT, 2048], BF16, isOutput=False)
    w2_e = dp("w2", [L, 128, 16, 512], BF16, isOutput=False)
    bkc_e = dp("bkc", [L, 128, NT], F32, isOutput=False)
    b1c_e = dp("b1c", [L, 128, 16], F32, isOutput=False)
    l1g_e = dp("l1g", [L, 128, NT], F32, isOutput=False)
    l1b_e = dp("l1b", [L, 128, NT], F32, isOutput=False)
    l2g_e = dp("l2g", [L, 128, NT], F32, isOutput=False)
    l2b_e = dp("l2b", [L, 128, NT], F32, isOutput=False)
    bvr_e = dp("bvr", [L, 1, 512], BF16, isOutput=False)
    bor_e = dp("bor", [L, 1, 512], BF16, isOutput=False)
    b2r_e = dp("b2r", [L, 1, 512], BF16, isOutput=False)
    gam_e = dp("gam", [1, L * H], F32, isOutput=False)
    nm1_e = dp("nm1", [NT, 128, 512], BF16, isOutput=False)
    nm0_e = dp("nm0", [NT, 128, 512], BF16, isOutput=False)
    pos_e = dp("posm", [NT, 128, 512], BF16, isOutput=False)
    tri_e = dp("tril", [128, 128], BF16, isOutput=False)
    onm_e = dp("onesm", [128, 128], BF16, isOutput=False)
    idn_e = dp("idn", [128, 128], F32, isOutput=False)
    out_e = dp("out", [BL, 2, NT, 128, 512], F32, isOutput=True)
    if DBG:
        dbg_an = dp("dbg_an", [128, NT, 512], F32, isOutput=True)
        dbg_x = dp("dbg_x", [128, NT, 512], F32, isOutput=True)
        dbg_e1 = dp("dbg_e1", [128, 512], F32, isOutput=True)
        dbg_e2 = dp("dbg_e2", [128, 512], F32, isOutput=True)
        dbg_v = dp("dbg_v", [128, 512], F32, isOutput=True)

    from contextlib import ExitStack
    with TileContext(nc) as tc, ExitStack() as _st:
        cst = _st.enter_context(tc.tile_pool(name="cst", bufs=1))
        wp = _st.enter_context(tc.tile_pool(name="wp", bufs=2))
        xp = _st.enter_context(tc.tile_pool(name="xp", bufs=1))
        apl = _st.enter_context(tc.tile_pool(name="apl", bufs=1))
        held = _st.enter_context(tc.tile_pool(name="held", bufs=8))
        eph = _st.enter_context(tc.tile_pool(name="eph", bufs=2))
        hp = _st.enter_context(tc.tile_pool(name="hp", bufs=2))
        rp = _st.enter_context(tc.tile_pool(name="rp", bufs=2))
        rs = _st.enter_context(tc.tile_pool(name="rs", bufs=1))
        lr = _st.enter_context(tc.tile_pool(name="lr", bufs=1))
        wq = _st.enter_context(tc.tile_pool(name="wq", bufs=2))
        ps = _st.enter_context(tc.tile_pool(name="ps", bufs=5, space="PSUM"))
        psb = _st.enter_context(tc.tile_pool(name="psb", bufs=2, space="PSUM"))
        psr = _st.enter_context(tc.tile_pool(name="psr", bufs=1, space="PSUM"))
        dma = nc.default_dma_engine

        nmt = {}
        posm = []
        for j in range(NT):
            nv, ic = NVS[j], 128 * j
            t1 = cst.tile([128, nv], BF16, tag=f"nm1_{j}")
            t0 = cst.tile([128, nv], BF16, tag=f"nm0_{j}")
            pm = cst.tile([128, nv], BF16, tag=f"pos_{j}")
            dma.dma_start(out=t1, in_=nm1_e[j, :, ic:512])
            dma.dma_start(out=t0, in_=nm0_e[j, :, ic:512])
            dma.dma_start(out=pm, in_=pos_e[j, :, ic:512])
            nmt[(1, j)] = t1
            nmt[(0, j)] = t0
            posm.append(pm)
        tril = cst.tile([128, 128], BF16, tag="tril")
        onesm = cst.tile([128, 128], BF16, tag="onesm")
        idn = cst.tile([128, 128], F32, tag="idn")
        dma.dma_start(out=tril, in_=tri_e[:])
        dma.dma_start(out=onesm, in_=onm_e[:])
        dma.dma_start(out=idn, in_=idn_e[:])
        onescol = cst.tile([128, 1], BF16, tag="onescol")
        ones1x = cst.tile([1, 128], BF16, tag="ones1x")
        onesrow = cst.tile([1, 512], BF16, tag="onesrow")
        ninv512 = cst.tile([1, 512], F32, tag="ninv512")
        nc.vector.memset(onescol, 1.0)
        nc.vector.memset(ones1x, 1.0)
        nc.vector.memset(onesrow, 1.0)
        nc.vector.memset(ninv512, -1.0 / 512.0)
        one11 = cst.tile([1, 1], F32, tag="one11")
        eps11 = cst.tile([1, 1], F32, tag="eps11")
        nc.vector.memset(one11, 1.0)
        nc.vector.memset(eps11, 1e-5)

        gam = cst.tile([1, L * H], F32, tag="gam")
        dma.dma_start(out=gam, in_=gam_e[:])
        ge = cst.tile([1, L * H], F32, tag="ge")
        nc.scalar.activation(out=ge, in_=gam, func=AF.Exp)
        gl = cst.tile([1, L * H], F32, tag="gl")
        nc.scalar.activation(out=gl, in_=ge, func=AF.Ln, bias=one11[:])
        gn = cst.tile([1, L * H], F32, tag="gn")
        nc.vector.tensor_scalar_mul(gn, gl, -1.0)
        mspg = cst.tile([128, L * H], F32, tag="mspg")
        nc.gpsimd.partition_broadcast(mspg[:], gn[:])

        xT = {'x': [], 'y': []}
        x16 = {'x': [], 'y': []}
        for b in range(BL):
            for st, src in (('x', x0_e), ('y', y0_e)):
                t = xp.tile([128, NT, 512], F32, tag=f"xT_{st}_{b}")
                dma.dma_start(out=t, in_=src[b])
                c = xp.tile([128, NT, 512], BF16, tag=f"x16_{st}_{b}")
                for tt in range(NT):
                    nc.vector.tensor_copy(out=c[:, tt, :], in_=t[:, tt, :])
                xT[st].append(t)
                x16[st].append(c)

        def layer_norm(b, pss, gcol, bcol, mu_bank, xtag, x16tag):
            # pss: list of 4 PSUM tiles holding centered x (mu already subtracted)
            # var = colsum(square)/512 ; returns (xnew f32, xnew16 bf16)
            sqs = []
            for et in range(NT):
                sq = hp.tile([128, 512], BF16, tag="sq")
                nc.scalar.activation(out=sq, in_=pss[et][:], func=AF.Square)
                sqs.append(sq)
            for et in range(NT):
                nc.tensor.matmul(out=mu_bank[32:33, :], lhsT=onescol[:], rhs=sqs[et][:],
                                 start=(et == 0), stop=(et == NT - 1))
            xc = []
            for et in range(NT):
                t = hp.tile([128, 512], F32, tag="xc")
                if et % 2 == 0:
                    nc.scalar.activation(out=t, in_=pss[et][:], func=AF.Copy)
                else:
                    nc.vector.tensor_copy(out=t, in_=pss[et][:])
                xc.append(t)
            r1 = rs.tile([1, 512], F32, tag="r1")
            nc.scalar.activation(out=r1, in_=mu_bank[32:33, :], func=AF.Ln,
                                 scale=1.0 / 512.0, bias=eps11[:])
            nc.scalar.activation(out=r1[:], in_=r1[:], func=AF.Exp, scale=-0.5)
            rstd16 = rs.tile([1, 512], BF16, tag="rstd16")
            nc.vector.tensor_copy(out=rstd16, in_=r1[:])
            rbl = hp.tile([128, 512], BF16, tag="rb")
            nc.gpsimd.partition_broadcast(rbl[:], rstd16[:])
            xo = xp.tile([128, NT, 512], F32, tag=xtag)
            xo16 = xp.tile([128, NT, 512], BF16, tag=x16tag)
            for et in range(NT):
                nc.vector.scalar_tensor_tensor(out=xo[:, et, :], in0=xc[et][:], scalar=1.0,
                                               op0=ALU.mult, in1=rbl[:], op1=ALU.mult)
                nc.scalar.activation(out=xo[:, et, :], in_=xo[:, et, :], func=AF.Identity,
                                     bias=bcol[:, et:et + 1], scale=gcol[:, et:et + 1])
                nc.vector.tensor_copy(out=xo16[:, et, :], in_=xo[:, et, :])
            return xo, xo16

        for l in range(NLAYERS):
            qs, vs, mf, ffn = LCFG[l]
            wk = wp.tile([128, NT, 512], BF16, tag="wk")
            wv = wp.tile([128, NT, 512], BF16, tag="wv")
            wo = wp.tile([128, NT, 512], BF16, tag="wo")
            dma.dma_start(out=wk, in_=wk_e[l])
            dma.dma_start(out=wv, in_=wv_e[l])
            dma.dma_start(out=wo, in_=wo_e[l])
            bkc = lr.tile([128, NT], F32, tag="bkc")
            dma.dma_start(out=bkc, in_=bkc_e[l])
            bvr = lr.tile([1, 512], BF16, tag="bvr")
            bor = lr.tile([1, 512], BF16, tag="bor")
            dma.dma_start(out=bvr, in_=bvr_e[l])
            dma.dma_start(out=bor, in_=bor_e[l])
            l1g = lr.tile([128, NT], F32, tag="l1g")
            l1b = lr.tile([128, NT], F32, tag="l1b")
            dma.dma_start(out=l1g, in_=l1g_e[l])
            dma.dma_start(out=l1b, in_=l1b_e[l])
            if ffn:
                b1c = lr.tile([128, 16], F32, tag="b1c")
                dma.dma_start(out=b1c, in_=b1c_e[l])
                b2r = lr.tile([1, 512], BF16, tag="b2r")
                dma.dma_start(out=b2r, in_=b2r_e[l])
                l2g = lr.tile([128, NT], F32, tag="l2g")
                l2b = lr.tile([128, NT], F32, tag="l2b")
                dma.dma_start(out=l2g, in_=l2g_e[l])
                dma.dma_start(out=l2b, in_=l2b_e[l])
                sb2 = lr.tile([1, 1], F32, tag="sb2")
                nc.vector.tensor_reduce(out=sb2, in_=b2r[:], axis=mybir.AxisListType.X,
                                        op=ALU.add)

            wobf = lr.tile([128, NT], F32, tag="wobf")
            for ct in range(NT):
                nc.vector.tensor_reduce(out=wobf[:, ct:ct + 1], in_=wo[:, ct, :],
                                        axis=mybir.AxisListType.X, op=ALU.add)
            wob16 = rs.tile([128, NT], BF16, tag="wob16")
            nc.vector.tensor_copy(out=wob16, in_=wobf)
            sbo = lr.tile([1, 1], F32, tag="sbo")
            nc.vector.tensor_reduce(out=sbo, in_=bor[:], axis=mybir.AxisListType.X,
                                    op=ALU.add)

            for b in range(BL):
                qin16 = x16[qs][b]
                vin16 = x16[vs][b]
                qk16 = apl.tile([128, NT, 512], BF16, tag="qk16")
                for et in range(NT):
                    pq = psb.tile([128, 512], F32, tag="pp")
                    for ct in range(NT):
                        nc.tensor.matmul(out=pq[:], lhsT=wk[:, ct, 128 * et:128 * et + 128],
                                         rhs=qin16[:, ct, :], start=(ct == 0), stop=(ct == NT - 1))
                    nc.scalar.activation(out=qk16[:, et, :], in_=pq[:], func=AF.Identity,
                                         bias=bkc[:, et:et + 1], scale=1.0)
                vh16 = apl.tile([128, NT, 512], BF16, tag="vh16")
                for jt in range(NT):
                    pv = psb.tile([128, 512], F32, tag="pp")
                    for ct in range(NT):
                        nc.tensor.matmul(out=pv[:], lhsT=vin16[:, ct, 128 * jt:128 * jt + 128],
                                         rhs=wv[:, ct, :], start=(ct == 0), stop=False)
                    nc.tensor.matmul(out=pv[:], lhsT=ones1x[:], rhs=bvr[:], start=False, stop=True)
                    nc.scalar.activation(out=vh16[:, jt, :], in_=pv[:], func=AF.Copy)

                # phase A
                sms, v16s = [], []
                TallA = rp.tile([128, 512], F32, tag="TallA")
                TallB = rp.tile([128, 512], F32, tag="TallB")
                Talls = (TallA, TallB)
                for h in range(H):
                    et, ro = h // 2, 64 * (h % 2)
                    e1h, smh, v16h = [], [], []
                    for jt in range(NT):
                        nv, ic = NVS[jt], 128 * jt
                        sps = ps.tile([128, 512], F32, tag="big")
                        nc.tensor.matmul(out=sps[:, 0:nv],
                                         lhsT=qk16[ro:ro + 64, et, 128 * jt:128 * jt + 128],
                                         rhs=qk16[ro:ro + 64, et, ic:512],
                                         start=True, stop=True)
                        sm = held.tile([128, nv], BF16, tag=f"sm{jt}")
                        nc.vector.scalar_tensor_tensor(out=sm[:], in0=sps[:, 0:nv], scalar=0.125,
                                                       in1=nmt[(mf, jt)][:],
                                                       op0=ALU.mult, op1=ALU.add)
                        e1 = eph.tile([128, nv], BF16, tag=f"e1{jt}")
                        nc.scalar.activation(out=e1, in_=sm, func=AF.Exp)
                        e1h.append(e1)
                        smh.append(sm)
                    v16h = []
                    for jt in range(NT):
                        nv, ic = NVS[jt], 128 * jt
                        Ep = ps.tile([128, 512], F32, tag="big")
                        for a in range(jt, NT):
                            nc.tensor.matmul(out=Ep[:, 128 * a:512],
                                             lhsT=(tril if a == jt else onesm)[:],
                                             rhs=e1h[a][:], start=(a == jt), stop=(a == NT - 1))
                        if jt == 0:
                            _tt = Talls[h // 4]
                            _pp = 32 * (h % 4)
                            nc.vector.tensor_tensor(out=_tt[_pp:_pp + 1, :], in0=Ep[0:1, 0:512],
                                                    in1=e1h[0][0:1, :], op=ALU.add)
                        v16 = held.tile([128, nv], BF16, tag=f"v16{jt}")
                        nc.vector.tensor_tensor(out=v16[:], in0=Ep[:, ic:512],
                                                in1=posm[jt][:], op=ALU.mult)
                        v16h.append(v16)
                    if DBG and l == 0 and b == 0 and h == 0:
                        _d = hp.tile([128, 512], F32, tag="xc")
                        nc.vector.tensor_copy(out=_d[:], in_=e1h[0][:])
                        dma.dma_start(out=dbg_e1[:], in_=_d[:])
                        _d2 = hp.tile([128, 512], F32, tag="xc")
                        nc.vector.tensor_copy(out=_d2[:], in_=v16h[0][:])
                        dma.dma_start(out=dbg_v[:], in_=_d2[:])
                    sms.append(smh)
                    v16s.append(v16h)
                # phase B (sqrt table) -- in-place sqrt
                for h in range(H):
                    for jt in range(NT):
                        nc.scalar.activation(out=v16s[h][jt][:], in_=v16s[h][jt][:], func=AF.Sqrt)
                nc.scalar.activation(out=TallA[:], in_=TallA[:], func=AF.Sqrt)
                nc.scalar.activation(out=TallB[:], in_=TallB[:], func=AF.Sqrt)
                # phase C (exp table)
                rT16s = []
                for _tt in Talls:
                    nc.vector.tensor_scalar_add(_tt[:], _tt[:], 1e-20)
                    _rT = rp.tile([128, 512], F32, tag="rT", name=f"rT_{id(_tt) % 97}")
                    nc.vector.reciprocal_approx_fast(out=_rT, in_=_tt[:])
                    rT16s.append(_rT)
                an16 = apl.tile([128, NT, 512], BF16, tag="an16")
                Zpair = None
                for h in range(H):
                    et = h // 2
                    rb = hp.tile([128, 512], BF16, tag="rb")
                    _r16 = rT16s[h // 4]
                    _pp = 32 * (h % 4)
                    _rrow = rs.tile([1, 512], BF16, tag="rTrow")
                    nc.vector.tensor_copy(out=_rrow[:], in_=_r16[_pp:_pp + 1, :])
                    nc.gpsimd.partition_broadcast(rb[:], _rrow[0:1, :])
                    e2h = []
                    for jt in range(NT):
                        nv, ic = NVS[jt], 128 * jt
                        u2 = v16s[h][jt]
                        nc.vector.tensor_tensor(out=u2[:], in0=u2[:], in1=rb[:, ic:512],
                                                op=ALU.mult)
                        nc.scalar.activation(out=u2[:], in_=u2[:], func=AF.Exp,
                                             scale=mspg[:, l * H + h:l * H + h + 1])
                        m2 = hp.tile([128, 512], BF16, tag="sq")
                        nc.vector.tensor_tensor(out=m2[:, 0:nv], in0=sms[h][jt], in1=u2[:],
                                                op=ALU.mult)
                        e2 = hp.tile([128, nv], BF16, tag=f"e2{jt}")
                        nc.scalar.activation(out=e2, in_=m2[:, 0:nv], func=AF.Exp)
                        e2h.append(e2)
                    Zp = psr.tile([64, 512], F32, tag="mu")
                    for a in range(NT):
                        nc.tensor.matmul(out=Zp[0:1, 128 * a:512], lhsT=onescol[:],
                                         rhs=e2h[a][:], start=(a == 0), stop=(a == NT - 1))
                    if h % 2 == 0:
                        Zpair = rs.tile([64, 512], F32, tag="Zpair")
                        atp = ps.tile([128, 512], F32, tag="big")
                    nc.vector.tensor_copy(out=Zpair[32 * (h % 2):32 * (h % 2) + 1, :], in_=Zp[0:1, :])
                    ro = 64 * (h % 2)
                    for a in range(NT):
                        nc.tensor.matmul(out=atp[ro:ro + 64, 128 * a:512],
                                         lhsT=vh16[:, a, 64 * h:64 * h + 64],
                                         rhs=e2h[a][:],
                                         start=(a == 0), stop=(a == NT - 1))
                    if DBG and l == 0 and b == 0 and h == 0:
                        _d3 = hp.tile([128, 512], F32, tag="xc")
                        nc.vector.tensor_copy(out=_d3[:], in_=e2h[0][:])
                        dma.dma_start(out=dbg_e2[:], in_=_d3[:])
                    if h % 2 == 1:
                        nc.vector.tensor_scalar_add(Zpair[:], Zpair[:], 1e-30)
                        rZ = rs.tile([64, 512], F32, tag="rZ")
                        nc.vector.reciprocal_approx_fast(out=rZ, in_=Zpair[:])
                        for par in range(2):
                            zr16 = rs.tile([1, 512], BF16, tag="zr")
                            nc.vector.tensor_copy(out=zr16[:], in_=rZ[32 * par:32 * par + 1, :])
                            rzb = hp.tile([128, 512], BF16, tag="rb")
                            nc.gpsimd.partition_broadcast(rzb[:], zr16[0:1, :])
                            rr = 64 * par
                            nc.vector.tensor_tensor(out=an16[rr:rr + 64, et, :],
                                                    in0=atp[rr:rr + 64, :],
                                                    in1=rzb[rr:rr + 64, :], op=ALU.mult)

                # Wo + bo + residual + (-mu) -> centered x in PSUM
                mup = psr.tile([64, 512], F32, tag="mu")
                for ct in range(NT):
                    nc.tensor.matmul(out=mup[0:1, :], lhsT=wob16[:, ct:ct + 1],
                                     rhs=an16[:, ct, :], start=(ct == 0), stop=False)
                for ct in range(NT):
                    nc.tensor.matmul(out=mup[0:1, :], lhsT=onescol[:], rhs=qin16[:, ct, :],
                                     start=False, stop=(ct == NT - 1))
                nmu = rs.tile([1, 512], BF16, tag="nmu")
                nc.vector.scalar_tensor_tensor(out=nmu, in0=mup[0:1, :], scalar=sbo[:],
                                               op0=ALU.add, in1=ninv512[:], op1=ALU.mult)
                o3p = []
                for et in range(NT):
                    op_ = ps.tile([128, 512], F32, tag="big")
                    for ct in range(NT):
                        nc.tensor.matmul(out=op_[:], lhsT=wo[:, ct, 128 * et:128 * et + 128],
                                         rhs=an16[:, ct, :], start=(ct == 0), stop=False)
                    nc.tensor.matmul(out=op_[:], lhsT=bor[0:1, 128 * et:128 * et + 128],
                                     rhs=onesrow[:], start=False, stop=False)
                    nc.tensor.matmul(out=op_[:], lhsT=idn[:], rhs=xT[qs][b][:, et, :],
                                     start=False, stop=False)
                    nc.tensor.matmul(out=op_[:], lhsT=ones1x[:], rhs=nmu[:], start=False, stop=True)
                    o3p.append(op_)
                if not ffn:
                    xn, xn16 = layer_norm(b, o3p, l1g, l1b, mup, f"xT_{qs}_{b}", f"x16_{qs}_{b}")
                    xT[qs][b] = xn
                    x16[qs][b] = xn16
                    continue
                xn, xn16 = layer_norm(b, o3p, l1g, l1b, mup, "ln1", "ln1b")
                if DBG and l == 0 and b == 0:
                    for _t in range(NT):
                        _d4 = hp.tile([128, 512], F32, tag="xc")
                        nc.vector.tensor_copy(out=_d4[:], in_=an16[:, _t, :])
                        dma.dma_start(out=dbg_an[:, _t, :].opt() if False else dbg_an[0:128, _t, :], in_=_d4[:])
                        nc.vector.tensor_copy(out=_d4[:], in_=xn[:, _t, :])
                        dma.dma_start(out=dbg_x[0:128, _t, :], in_=_d4[:])

                # FFN (hidden + weights streamed per quarter)
                mu2 = psr.tile([64, 512], F32, tag="mu")
                o4p = [ps.tile([128, 512], F32, tag="big", name=f"o4p{_i}") for _i in range(NT)]
                for fq in range(8):
                    w1q = wq.tile([128, NT, 256], BF16, tag="w1q")
                    dma.dma_start(out=w1q, in_=w1_e[l, :, :, 256 * fq:256 * fq + 256])
                    w2q = wq.tile([128, 2, 512], BF16, tag="w2q")
                    dma.dma_start(out=w2q, in_=w2_e[l, :, 2 * fq:2 * fq + 2, :])
                    w2bq = rs.tile([128, 2], F32, tag="w2bq")
                    for fi in range(2):
                        nc.vector.tensor_reduce(out=w2bq[:, fi:fi + 1], in_=w2q[:, fi, :],
                                                axis=mybir.AxisListType.X, op=ALU.add)
                    w2bq16 = rs.tile([128, 2], BF16, tag="w2bq16")
                    nc.vector.tensor_copy(out=w2bq16, in_=w2bq)
                    for fi in range(2):
                        ft = 2 * fq + fi
                        php = psb.tile([128, 512], F32, tag="pp")
                        for ct in range(NT):
                            nc.tensor.matmul(out=php[:], lhsT=w1q[:, ct, 128 * fi:128 * fi + 128],
                                             rhs=xn16[:, ct, :], start=(ct == 0), stop=(ct == NT - 1))
                        h16f = hp.tile([128, 512], BF16, tag="h16f")
                        nc.scalar.activation(out=h16f, in_=php[:], func=AF.Relu,
                                             bias=b1c[:, ft:ft + 1], scale=1.0)
                        nc.tensor.matmul(out=mu2[0:1, :], lhsT=w2bq16[:, fi:fi + 1],
                                         rhs=h16f[:], start=(ft == 0), stop=False)
                        for et in range(NT):
                            nc.tensor.matmul(out=o4p[et][:], lhsT=w2q[:, fi, 128 * et:128 * et + 128],
                                             rhs=h16f[:], start=(ft == 0), stop=False)
                for ct in range(NT):
                    nc.tensor.matmul(out=mu2[0:1, :], lhsT=onescol[:], rhs=xn16[:, ct, :],
                                     start=False, stop=(ct == NT - 1))
                nmu2 = rs.tile([1, 512], BF16, tag="nmu")
                nc.vector.scalar_tensor_tensor(out=nmu2, in0=mu2[0:1, :], scalar=sb2[:],
                                               op0=ALU.add, in1=ninv512[:], op1=ALU.mult)
                for et in range(NT):
                    nc.tensor.matmul(out=o4p[et][:], lhsT=b2r[0:1, 128 * et:128 * et + 128],
                                     rhs=onesrow[:], start=False, stop=False)
                    nc.tensor.matmul(out=o4p[et][:], lhsT=idn[:], rhs=xn[:, et, :],
                                     start=False, stop=False)
                    nc.tensor.matmul(out=o4p[et][:], lhsT=ones1x[:], rhs=nmu2[:], start=False, stop=True)
                xo, xo16 = layer_norm(b, o4p, l2g, l2b, mu2, f"xT_{qs}_{b}", f"x16_{qs}_{b}")
                xT[qs][b] = xo
                x16[qs][b] = xo16

        for b in range(BL):
            for oi, st in enumerate(['x', 'y']):
                src = xT[st][b]
                for it in range(NT):
                    tok = hp.tile([128, 512], F32, tag="xc")
                    for et in range(NT):
                        pt = psb.tile([128, 128], F32, tag="pp")
                        nc.tensor.transpose(out=pt[:], in_=src[:, et, 128 * it:128 * it + 128],
                                            identity=idn[:])
                        nc.scalar.activation(out=tok[:, 128 * et:128 * et + 128], in_=pt[:],
                                             func=AF.Copy)
                    dma.dma_start(out=out_e[b, oi, it], in_=tok[:])

    nc.compile()
    return nc


def _prep(inputs):
    f32 = np.float32
    q = np.asarray(inputs['q_embed_data'], f32)
    qa = np.asarray(inputs['qa_embed_data'], f32)

    def fmaj(x):
        # [L, R, C] -> [L, 128, R//128, C] partition-major on R
        Lx, R, C = x.shape
        return np.ascontiguousarray(x.reshape(Lx, R // 128, 128, C).transpose(0, 2, 1, 3))

    def cols(v):
        return np.ascontiguousarray(np.asarray(v, f32).reshape(L, -1, 128).transpose(0, 2, 1))

    jj = np.arange(S)[:, None]   # j (partition/rows)
    ii = np.arange(S)[None, :]   # i (free/cols)
    nm1 = np.where(jj <= ii, 0.0, NEG).astype(BF).reshape(NT, 128, S)
    nm0 = np.where(jj < ii, 0.0, NEG).astype(BF).reshape(NT, 128, S)
    posm = np.abs(ii - jj).astype(f32).astype(BF).reshape(NT, 128, S)

    base = {
        'wk': fmaj(np.asarray(inputs['Wk'], f32)).astype(BF),
        'wv': fmaj(np.asarray(inputs['Wv'], f32)).astype(BF),
        'wo': fmaj(np.asarray(inputs['Wo'], f32)).astype(BF),
        'w1': fmaj(np.asarray(inputs['W1'], f32)).astype(BF),
        'w2': fmaj(np.asarray(inputs['W2'], f32)).astype(BF),
        'bkc': cols(inputs['bk']),
        'b1c': cols(inputs['b1']),
        'l1g': cols(inputs['ln1_g']), 'l1b': cols(inputs['ln1_b']),
        'l2g': cols(inputs['ln2_g']), 'l2b': cols(inputs['ln2_b']),
        'bvr': np.asarray(inputs['bv'], f32).astype(BF).reshape(L, 1, S),
        'bor': np.asarray(inputs['bo'], f32).astype(BF).reshape(L, 1, S),
        'b2r': np.asarray(inputs['b2'], f32).astype(BF).reshape(L, 1, S),
        'gam': np.asarray(inputs['gammas'], f32).reshape(1, L * H),
        'nm1': np.ascontiguousarray(nm1), 'nm0': np.ascontiguousarray(nm0),
        'posm': np.ascontiguousarray(posm),
        'tril': np.tril(np.ones((128, 128), f32), -1).astype(BF),
        'onesm': np.ones((128, 128), f32).astype(BF),
        'idn': np.eye(128, dtype=f32),
    }
    in_maps = []
    for c in range(NCORES):
        m = dict(base)
        for k, src in (('x0', q), ('y0', qa)):
            sh = src[c * BL:(c + 1) * BL]  # [BL, S(i), D(c)]
            m[k] = np.ascontiguousarray(
                sh.transpose(0, 2, 1).reshape(BL, NT, 128, S).transpose(0, 2, 1, 3))
        in_maps.append(m)
    return in_maps


def kernel(**inputs):
    if 'nc' not in _CACHE:
        _CACHE['nc'] = _build()
    nc = _CACHE['nc']
    in_maps = _prep(inputs)
    res = run_bass_kernel_spmd(nc, in_maps, list(range(NCORES)))
    xs, ys = [], []
    for c in range(NCORES):
        o = np.asarray(res.results[c]['out'])  # [BL, 2, NT, 128, 512]
        xs.append(o[:, 0].reshape(BL, S, D))
        ys.append(o[:, 1].reshape(BL, S, D))
    return (np.concatenate(xs, 0), np.concatenate(ys, 0))

